# revision 1
# baseline (speedup 1.0000x reference)
"""Trainium2 Bass kernel for AlphaFold-style gated row attention.

Reference math (per MSA row r, B=1, R=128, Q=K=256, C=256, H=8, D=32):
    q = q_x @ Wq; k = k_x @ Wk; v = v_x @ Wv            (per-head D=32)
    a = softmax(q k^T / sqrt(D) + bias)                  (per head)
    o = (a @ v) * sigmoid(q_x @ Wg + bg)
    out = o @ Wo + bo

Sharding: 128 rows data-parallel over 8 NeuronCores (16 rows/core), weights
and pair bias replicated. No collectives needed.

Device strategy (all matmuls bf16, f32 PSUM accumulate):
  - Host pre-transposes activations (x^T layouts) so every GEMM contraction
    dim lands on SBUF partitions; host also pre-scales Wq by 1/sqrt(D),
    pre-transposes the pair bias, and prepares broadcast constants.
  - Logits computed transposed per head: aT[k, q] (softmax sums become
    col-tiled ones-matmuls on the TensorEngine; exp has no max-subtraction,
    logits are O(5) so f32 exp is safe).
  - Pair bias injected into PSUM via identity-matmuls before logit matmuls
    accumulate on top.
  - Gate uses tanh (same ACT table set as exp): sigmoid(x) = 0.5*(1+tanh(x/2)),
    with the 0.5 folded into the softmax-sum reciprocal (sums scaled by 2).
  - Per-head softmax denominators come out of col-tiled matmuls directly in
    the layout needed to scale the attention output (no partition broadcast).
"""
import numpy as np
import ml_dtypes

import concourse.bass as bass
import concourse.tile as tile
from concourse import bacc, mybir
from concourse.bass_utils import run_bass_kernel_spmd

BF16 = mybir.dt.bfloat16
F32 = mybir.dt.float32
AF = mybir.ActivationFunctionType
ALU = mybir.AluOpType

N_CORES = 8
R_LOC = 16          # rows per core
QS = 256            # query length
KS = 256            # key length
CH = 256            # channels
H = 8               # heads
D = 32              # head dim
NORM = 1.0 / np.sqrt(D)

nbf = ml_dtypes.bfloat16


def build_nc(reps=1, eb_bias=False, dma_chunks=4, work_bufs=3, pp_bufs=4, av_inline=True, out_rows=4, qk_act=False):
    nc = bacc.Bacc("TRN2", target_bir_lowering=False, debug=False,
                   num_devices=N_CORES)

    def din(name, shape, dt=BF16):
        return nc.dram_tensor(name, shape, dt, kind="ExternalInput").ap()

    # Host-prepped layouts (see prep_core_inputs for exact index maps).
    qxT = din("qxT", [128, R_LOC * 512])     # [p, 512r + 256cc + s]
    kxT = din("kxT", [128, R_LOC * 512])
    vxT = din("vxT", [128, R_LOC * 512])
    biasTI = din("biasTI", [128, 4096])      # [p, 2048kc + 1024hg + 256j + q]
    wq = din("wq", [128, 512])               # [p, 256cc + hd] (norm folded)
    wk = din("wk", [128, 512])
    wg = din("wg", [128, 512])
    wv = din("wv", [128, 512])
    wo = din("wo", [128, 512])               # [p, 256hc + c]
    bo_bc = din("bo_bc", [128, 512], F32)    # [p, 256qc + c] = bo[c]
    bgh = din("bgh", [128, 2], F32)          # 0.5*bg per hc
    twos = din("twos", [128, 32])            # 2.0 (sums lhsT; folds 0.5 gate)
    ident = din("ident", [128, 128])         # I128 (bias injection)
    eb = din("eb", [128, 4096])              # exp(bias) in aT layout

    out = nc.dram_tensor("out", [128, R_LOC * 512], BF16,
                         kind="ExternalOutput").ap()

    with tile.TileContext(nc) as tc:
        import contextlib
        with contextlib.ExitStack() as ctx:
            const = ctx.enter_context(tc.tile_pool(name="const", bufs=1))
            work = ctx.enter_context(tc.tile_pool(name="work", bufs=work_bufs))
            outp = ctx.enter_context(tc.tile_pool(name="outp", bufs=2))
            psL = ctx.enter_context(
                tc.tile_pool(name="psL", bufs=2, space="PSUM"))
            psB = ctx.enter_context(
                tc.tile_pool(name="psB", bufs=pp_bufs, space="PSUM"))

            # ---- persistent loads ----
            def load(ap, dt=BF16, tag=None):
                t = const.tile(list(ap.shape), dt, tag=tag or ap.tensor.name)
                nc.sync.dma_start(t[:], ap)
                return t

            def load_chunked(ap, n):
                t = const.tile(list(ap.shape), BF16, tag=ap.tensor.name)
                w = ap.shape[1] // n
                for i in range(n):
                    nc.sync.dma_start(t[:, i * w:(i + 1) * w],
                                      ap[:, i * w:(i + 1) * w])
                return t

            qxT_sb = load_chunked(qxT, dma_chunks)
            kxT_sb = load_chunked(kxT, dma_chunks)
            vxT_sb = load_chunked(vxT, dma_chunks)
            biasTI_sb = load(biasTI)
            wq_sb = load(wq)
            wk_sb = load(wk)
            wg_sb = load(wg)
            wv_sb = load(wv)
            wo_sb = load(wo)
            bo_sb = load(bo_bc, F32)
            bgh_sb = load(bgh, F32)
            twos_sb = load(twos)
            ident_sb = load(ident)
            eb_sb = load(eb)

            def xslice(t, r, cc):
                return t[:, r * 512 + cc * 256: r * 512 + (cc + 1) * 256]

            out_batch = None
            for rep in range(reps):
              for r in range(R_LOC):
                  rr = r % out_rows
                  if rr == 0:
                      out_batch = outp.tile([128, out_rows * 512], BF16, tag="ob")

                  # ---- projections (PE) ----
                  qk = []
                  for hc in range(2):
                      t = psB.tile([128, 512], F32, tag="pp")
                      qk.append(t)
                      for cc in range(2):
                          nc.tensor.matmul(
                              t[:, 0:256],
                              wq_sb[:, cc * 256 + hc * 128: cc * 256 + hc * 128 + 128],
                              xslice(qxT_sb, r, cc),
                              start=(cc == 0), stop=False)
                          nc.tensor.matmul(
                              t[:, 256:512],
                              wk_sb[:, cc * 256 + hc * 128: cc * 256 + hc * 128 + 128],
                              xslice(kxT_sb, r, cc),
                              start=False, stop=(cc == 1))
                  g = psB.tile([128, 512], F32, tag="pp")
                  for hc in range(2):
                      for cc in range(2):
                          nc.tensor.matmul(
                              g[:, hc * 256:(hc + 1) * 256],
                              wg_sb[:, cc * 256 + hc * 128: cc * 256 + hc * 128 + 128],
                              xslice(qxT_sb, r, cc),
                              start=(hc == 0 and cc == 0),
                              stop=(hc == 1 and cc == 1))
                  v = psB.tile([128, 512], F32, tag="pp")
                  for kc in range(2):
                      for cc in range(2):
                          nc.tensor.matmul(
                              v[:, kc * 256:(kc + 1) * 256],
                              xslice(vxT_sb, r, cc)[:, kc * 128:(kc + 1) * 128],
                              wv_sb[:, cc * 256:(cc + 1) * 256],
                              start=(kc == 0 and cc == 0),
                              stop=(kc == 1 and cc == 1))

                  # ---- PSUM -> SBUF casts (DVE) + gate tanh (ACT) ----
                  qkT = []
                  for hc in range(2):
                      t = work.tile([128, 512], BF16, tag=f"qkT{hc}")
                      if qk_act:
                          nc.scalar.copy(t[:], qk[hc][:])
                      else:
                          nc.vector.tensor_copy(t[:], qk[hc][:])
                      qkT.append(t)
                  v_sb = work.tile([128, 512], BF16, tag="v_sb")
                  nc.vector.tensor_copy(v_sb[:], v[:])
                  tanhT = work.tile([128, 512], BF16, tag="tanhT")
                  for hc in range(2):
                      nc.scalar.activation(
                          tanhT[:, hc * 256:(hc + 1) * 256],
                          g[:, hc * 256:(hc + 1) * 256],
                          AF.Tanh, scale=0.5, bias=bgh_sb[:, hc:hc + 1])

                  # ---- logits + exp per head-pair chunk ----
                  # Each 2KB psum bank holds ONE head (kc0|kc1) so every
                  # row-tiled matmul in a bank shares its row group (HW
                  # constraint: mixed row-groups per bank fault). Bias is
                  # injected by a full-row identity matmul (exempt).
                  aT = work.tile([128, 4096], BF16, tag="aT")
                  for hp in range(4):
                      lg = psL.tile([128, 1024], F32, tag="lg")
                      for dh in range(2):
                          h = 2 * hp + dh
                          hg, j = h // 4, h % 4
                          if not eb_bias:
                              nc.tensor.matmul(
                                  lg[:, dh * 512:(dh + 1) * 512],
                                  ident_sb[:],
                                  biasTI_sb[:, 512 * h: 512 * h + 512],
                                  start=True, stop=False)
                          for kc in range(2):
                              nc.tensor.matmul(
                                  lg[:, dh * 512 + kc * 256: dh * 512 + kc * 256 + 256],
                                  qkT[hg][32 * j:32 * j + 32,
                                          256 + kc * 128: 256 + kc * 128 + 128],
                                  qkT[hg][32 * j:32 * j + 32, 0:256],
                                  start=(eb_bias and kc == 0), stop=(kc == 1),
                                  tile_position=(32 * j, 0))
                      if eb_bias:
                          araw = work.tile([128, 1024], BF16, tag="araw")
                          nc.scalar.activation(araw[:], lg[:], AF.Exp)
                          nc.vector.tensor_mul(
                              aT[:, 1024 * hp: 1024 * (hp + 1)], araw[:],
                              eb_sb[:, 1024 * hp: 1024 * (hp + 1)])
                      else:
                          nc.scalar.activation(
                              aT[:, 1024 * hp: 1024 * (hp + 1)], lg[:], AF.Exp)
                      if hp == 0:
                          oT = psB.tile([128, 512], F32, tag="pp")
                          sums = psB.tile([128, 512], F32, tag="pp")
                      if av_inline:
                          for h in (2 * hp, 2 * hp + 1):
                              hc, j = h // 4, h % 4
                              for kc in range(2):
                                  nc.tensor.matmul(
                                      oT[32 * j:32 * j + 32, hc * 256:(hc + 1) * 256],
                                      v_sb[:, kc * 256 + 32 * h: kc * 256 + 32 * h + 32],
                                      aT[:, 512 * h + 256 * kc: 512 * h + 256 * kc + 256],
                                      start=(kc == 0), stop=(kc == 1),
                                      tile_position=(0, 32 * j),
                                      skip_group_check=True)
                                  nc.tensor.matmul(
                                      sums[32 * j:32 * j + 32, hc * 256:(hc + 1) * 256],
                                      twos_sb[:, 0:32],
                                      aT[:, 512 * h + 256 * kc: 512 * h + 256 * kc + 256],
                                      start=(kc == 0), stop=(kc == 1),
                                      tile_position=(0, 32 * j),
                                      skip_group_check=True)
                  if not av_inline:
                      for kc in range(2):
                          for h in range(H):
                              hc, j = h // 4, h % 4
                              nc.tensor.matmul(
                                  oT[32 * j:32 * j + 32, hc * 256:(hc + 1) * 256],
                                  v_sb[:, kc * 256 + 32 * h: kc * 256 + 32 * h + 32],
                                  aT[:, 512 * h + 256 * kc: 512 * h + 256 * kc + 256],
                                  start=(kc == 0 and h < 4),
                                  stop=(kc == 1 and h >= 4),
                                  tile_position=(0, 32 * j),
                                  skip_group_check=True)
                              nc.tensor.matmul(
                                  sums[32 * j:32 * j + 32, hc * 256:(hc + 1) * 256],
                                  twos_sb[:, 0:32],
                                  aT[:, 512 * h + 256 * kc: 512 * h + 256 * kc + 256],
                                  start=(kc == 0 and h < 4),
                                  stop=(kc == 1 and h >= 4),
                                  tile_position=(0, 32 * j),
                                  skip_group_check=True)

                  # ---- normalize + gate (DVE) ----
                  recip = work.tile([128, 512], F32, tag="recip")
                  nc.vector.reciprocal_approx_fast(recip[:], sums[:])
                  oS = work.tile([128, 512], BF16, tag="oS")
                  nc.vector.tensor_mul(oS[:], oT[:], recip[:])
                  gated = work.tile([128, 512], BF16, tag="gated")
                  nc.vector.scalar_tensor_tensor(
                      gated[:], tanhT[:], 1.0, oS[:], ALU.add, ALU.mult)

                  # ---- output projection + bias (PE + DVE) ----
                  op = psB.tile([128, 512], F32, tag="pp")
                  for qc in range(2):
                      for hc in range(2):
                          nc.tensor.matmul(
                              op[:, qc * 256:(qc + 1) * 256],
                              gated[:, hc * 256 + qc * 128: hc * 256 + qc * 128 + 128],
                              wo_sb[:, hc * 256:(hc + 1) * 256],
                              start=(qc == 0 and hc == 0),
                              stop=(qc == 1 and hc == 1))
                  nc.vector.scalar_tensor_tensor(
                      out_batch[:, rr * 512:(rr + 1) * 512],
                      op[:], 1.0, bo_sb[:], ALU.mult, ALU.add)

                  if rr == 3:
                      nc.sync.dma_start(
                          out[:, (r - 3) * 512: (r + 1) * 512], out_batch[:])

    nc.compile()
    return nc


def prep_core_inputs(q_x, k_x, v_x, bias, Wq, Wk, Wv, Wo, bo, Wg, bg):
    """Build per-core input maps. q_x/k_x/v_x: [128, 256, 256] f32 (batch
    squeezed); bias: [8, 256, 256]; weights as in reference."""
    def xT_prep(x):  # [16,256,256] (r,s,c) -> [128, 16*512] bf16
        a = x.reshape(R_LOC, QS, 2, 128).transpose(3, 0, 2, 1)
        return np.ascontiguousarray(a.reshape(128, R_LOC * 512)).astype(nbf)

    def w_prep(w):   # [256,256] -> [128, 512]
        return np.ascontiguousarray(
            w.reshape(2, 128, 256).transpose(1, 0, 2).reshape(128, 512)
        ).astype(nbf)

    # bias [8(h),256(q),256(k)] -> [p, 512h + 256kc + q]
    b = bias.reshape(H, QS, 2, 128)              # [h, q, kc, p]
    b = b.transpose(3, 0, 2, 1)                  # [p, h, kc, q]
    biasTI = np.ascontiguousarray(b.reshape(128, 4096)).astype(nbf)

    eb = np.ascontiguousarray(
        np.exp(b.astype(np.float32)).reshape(128, 4096)).astype(nbf)

    shared = {
        "biasTI": biasTI,
        "eb": eb,
        "wq": w_prep(Wq * NORM),
        "wk": w_prep(Wk),
        "wg": w_prep(Wg),
        "wv": w_prep(Wv),
        "wo": w_prep(Wo),
        "bo_bc": np.ascontiguousarray(
            np.tile(bo.astype(np.float32), (128, 2))),
        "bgh": np.ascontiguousarray(
            (0.5 * bg.astype(np.float32)).reshape(2, 128).T),
        "twos": np.full((128, 32), 2.0, dtype=nbf),
        "ident": np.eye(128, dtype=np.float32).astype(nbf),
    }
    in_maps = []
    for c in range(N_CORES):
        sl = slice(c * R_LOC, (c + 1) * R_LOC)
        m = dict(shared)
        m["qxT"] = xT_prep(q_x[sl])
        m["kxT"] = xT_prep(k_x[sl])
        m["vxT"] = xT_prep(v_x[sl])
        in_maps.append(m)
    return in_maps


def assemble_output(results):
    """results: list of per-core dicts with 'out' [128, 8192] bf16."""
    full = np.empty((128, QS, CH), dtype=np.float32)
    for c in range(N_CORES):
        o = np.asarray(results[c]["out"]).astype(np.float32)
        o = o.reshape(128, R_LOC, 2, 256).transpose(1, 2, 0, 3)
        full[c * R_LOC:(c + 1) * R_LOC] = o.reshape(R_LOC, QS, CH)
    return full.reshape(1, 128, QS, CH)


_CACHE = {}


def _get_nc():
    if "nc" not in _CACHE:
        _CACHE["nc"] = build_nc()
    return _CACHE["nc"]


def kernel(q_x, k_x, v_x, bias, Wq, Wk, Wv, Wo, bo, Wg, bg):
    q_x = np.asarray(q_x, dtype=np.float32).reshape(128, QS, CH)
    k_x = np.asarray(k_x, dtype=np.float32).reshape(128, KS, CH)
    v_x = np.asarray(v_x, dtype=np.float32).reshape(128, KS, CH)
    bias = np.asarray(bias, dtype=np.float32).reshape(H, QS, KS)
    args = [np.asarray(a, dtype=np.float32)
            for a in (Wq, Wk, Wv, Wo, bo, Wg, bg)]
    nc = _get_nc()
    in_maps = prep_core_inputs(q_x, k_x, v_x, bias, *args)
    res = run_bass_kernel_spmd(nc, in_maps, core_ids=list(range(N_CORES)))
    return assemble_output(res.results)

